# revision 15
# baseline (speedup 1.0000x reference)
"""GAT (2-layer, PyG-style) Trainium2 Bass kernel — 8-core SPMD.

Strategy (dst-sharded graph parallel, per the sharding hint):
  - Nodes padded to a multiple of 128*ncores; core c owns LT=20 node tiles
    (rows [2560c, 2560c+2560)).  Edges (with self-loops) sorted by dst and
    assigned to the dst owner; per node-tile edge lists are padded to
    multiples of 128 ("chunks"), chunk counts unified across cores so one
    SPMD program fits all.
  - Per layer (one SPMD launch each):
      proj: every core computes h = x @ [W | W@A_src | W@A_dst] for ALL
            nodes (psum cols CO.. give per-node logit pieces al_s/al_d for
            free) and writes rows [h | al_s | al_d | pad] (768B) to a
            private DRAM table.
      edge: per <=512-edge group, dma_gather pulls source rows by src id;
            one-hot matmuls (iota vs per-edge local-dst) expand al_d per
            edge and aggregate exp-weighted messages per dst tile in PSUM,
            with extra columns accumulating the softmax denominator.
            Epilogue: divide by denominator, +bias (+ReLU for layer 1).
  - Softmax is computed without the max-subtraction (logits are O(1) here,
    exp is safe in fp32); out = (sum_e exp_e * h_src) / sum_e exp_e.
  - Host assembles layer-1 shards and restages for layer 2.
"""

import os
import sys
from contextlib import ExitStack

import numpy as np

for _p in ("/opt/trn_rl_repo",):
    if os.path.isdir(_p) and _p not in sys.path:
        sys.path.insert(0, _p)

import ml_dtypes  # noqa: E402

from concourse import bacc, bass, tile  # noqa: E402
import concourse.mybir as mybir  # noqa: E402
from concourse.bass_utils import run_bass_kernel_spmd  # noqa: E402

F32 = mybir.dt.float32
BF16 = mybir.dt.bfloat16
I16 = mybir.dt.int16
BF = ml_dtypes.bfloat16
OP = mybir.AluOpType
AF = mybir.ActivationFunctionType

NEG_SLOPE = 0.2


class Cfg:
    def __init__(self, n_nodes, ch_in, ch_out, heads, ncores):
        self.N = n_nodes
        self.CH = ch_in
        self.CO = ch_out
        self.H = heads
        self.NC = ncores
        self.PT = 128
        gt_raw = -(-n_nodes // 128)
        self.LT = -(-gt_raw // ncores)      # local node tiles per core
        self.GT = self.LT * ncores          # global tiles (padded)
        self.NPAD = self.GT * 128
        self.BLK = self.LT * 128            # node rows per core
        self.AL = self.CO + 2 * self.H      # used table-row columns
        self.ROW = -(-self.AL // 128) * 128  # table row width (bf16 elems)
        self.GROUP = 4                      # chunks per DVE group
        self.KIN = ch_in // 128


# --------------------------------------------------------------------------
# host-side edge plan (shared by both layers)
# --------------------------------------------------------------------------
def build_plan(cfg: Cfg, src: np.ndarray, dst: np.ndarray):
    NC, LT, BLK, PT = cfg.NC, cfg.LT, cfg.BLK, cfg.PT
    order = np.argsort(dst, kind="stable")
    src = np.asarray(src)[order].astype(np.int64)
    dst = np.asarray(dst)[order].astype(np.int64)

    counts = np.zeros((NC, LT), np.int64)
    seg = {}
    for c in range(NC):
        lo = np.searchsorted(dst, BLK * c)
        hi = np.searchsorted(dst, BLK * (c + 1))
        dl = dst[lo:hi] - BLK * c
        sl = src[lo:hi]
        for t in range(LT):
            a = np.searchsorted(dl, PT * t)
            b = np.searchsorted(dl, PT * (t + 1))
            counts[c, t] = b - a
            seg[(c, t)] = (sl[a:b], dl[a:b] - PT * t)

    chunks = [max(1, int(-(-counts[:, t].max() // PT))) for t in range(LT)]
    ecore = PT * int(np.sum(chunks))

    # group schedule: (tile, chunk0(global), loc0(in-tile), L, first, last)
    sched = []
    cum = 0
    for t in range(LT):
        k = chunks[t]
        j = 0
        while j < k:
            L = min(cfg.GROUP, k - j)
            sched.append((t, cum + j, j, L, j == 0, j + L == k))
            j += L
        cum += k
    assert len(sched) <= 128, f"too many groups: {len(sched)}"

    gidx = np.zeros((NC, 128, ecore // 16), np.int16)
    gidx2 = np.zeros((NC, 128, ecore // 16), np.int16)
    dstp = np.full((NC, 128, ecore // PT), -1.0, np.float32)
    for c in range(NC):
        s_full = np.zeros(ecore, np.int64)
        g_full = np.zeros(ecore, np.int64)
        d_full = np.full(ecore, -1.0, np.float32)
        off = 0
        for t in range(LT):
            k = int(counts[c, t])
            s_full[off:off + k] = seg[(c, t)][0]
            d_full[off:off + k] = seg[(c, t)][1]
            g_full[off:off + k] = seg[(c, t)][1] + BLK * c + PT * t
            off += PT * chunks[t]
        gidx[c] = np.tile(s_full.astype(np.int16).reshape(-1, 16).T, (8, 1))
        gidx2[c] = np.tile(g_full.astype(np.int16).reshape(-1, 16).T, (8, 1))
        dstp[c] = d_full.reshape(-1, PT).T

    cumstart = np.concatenate([[0], np.cumsum(chunks)]).astype(int)
    return dict(chunks=chunks, ecore=ecore, gidx=gidx, gidx2=gidx2,
                dstp=dstp, sched=sched, cumstart=cumstart)


# --------------------------------------------------------------------------
# device program for one GAT layer
# --------------------------------------------------------------------------
def build_layer_program(cfg: Cfg, plan, relu: bool):
    PT, CO, H, LT, GT = cfg.PT, cfg.CO, cfg.H, cfg.LT, cfg.GT
    AL, ROW, GROUP, KIN = cfg.AL, cfg.ROW, cfg.GROUP, cfg.KIN
    CPH = CO // H
    ecore = plan["ecore"]
    sched = plan["sched"]

    nc = bacc.Bacc("TRN2", target_bir_lowering=False, debug=False,
                   num_devices=cfg.NC)

    xT = nc.dram_tensor("xT", [cfg.CH, cfg.NPAD], BF16, kind="ExternalInput")
    wext = nc.dram_tensor("wext", [128, KIN, AL], BF16, kind="ExternalInput")
    bias = nc.dram_tensor("bias", [128, CO], F32, kind="ExternalInput")
    iotar = nc.dram_tensor("iotar", [128, 128], BF16, kind="ExternalInput")
    gidx_d = nc.dram_tensor("gidx", [128, ecore // 16], I16,
                            kind="ExternalInput")
    gidx2_d = nc.dram_tensor("gidx2", [128, ecore // 16], I16,
                             kind="ExternalInput")
    dstp_d = nc.dram_tensor("dstp", [128, ecore // PT], BF16,
                            kind="ExternalInput")
    out_d = nc.dram_tensor("out", [cfg.BLK, CO], F32, kind="ExternalOutput")

    HB = next(h for h in (8, 4, 2, 1) if GT % h == 0)  # tiles per write batch

    with tile.TileContext(nc) as tc, ExitStack() as ctx:
        consts = ctx.enter_context(tc.tile_pool(name="consts", bufs=1))
        xpool = ctx.enter_context(tc.tile_pool(name="xp", bufs=3))
        hpool = ctx.enter_context(tc.tile_pool(name="hp", bufs=2))
        gpool = ctx.enter_context(tc.tile_pool(name="gp", bufs=4))
        epool = ctx.enter_context(tc.tile_pool(name="ep", bufs=3))
        opool = ctx.enter_context(tc.tile_pool(name="op", bufs=1))
        dpool = ctx.enter_context(tc.tile_pool(name="dram", bufs=1,
                                               space="DRAM"))
        pproj = ctx.enter_context(tc.tile_pool(name="pproj", bufs=2,
                                               space="PSUM"))
        pagg = ctx.enter_context(tc.tile_pool(name="pagg", bufs=2,
                                              space="PSUM"))

        htab = dpool.tile([cfg.NPAD, ROW], BF16)

        # ---- constants ----
        w_t = consts.tile([128, KIN, AL], BF16)
        nc.sync.dma_start(out=w_t[:], in_=wext[:])
        bias_t = consts.tile([128, CO], F32)
        nc.sync.dma_start(out=bias_t[:], in_=bias[:])
        iotar_t = consts.tile([128, 1, 128], BF16)
        nc.sync.dma_start(out=iotar_t[:], in_=iotar[:])
        gidx_t = consts.tile([128, ecore // 16], I16)
        nc.sync.dma_start(out=gidx_t[:], in_=gidx_d[:])
        gidx2_t = consts.tile([128, ecore // 16], I16)
        nc.sync.dma_start(out=gidx2_t[:], in_=gidx2_d[:])
        dstp_t = consts.tile([128, ecore // PT, 1], BF16)
        nc.sync.dma_start(out=dstp_t[:], in_=dstp_d[:])

        # ---- projection: build the full h-table ----
        xT_v = xT[:].rearrange("(k p) n -> p k n", p=128)
        for b in range(GT // HB):
            xt_t = xpool.tile([128, KIN, HB * 128], BF16, tag="xt")
            nc.sync.dma_start(
                out=xt_t[:], in_=xT_v[:, :, b * HB * 128:(b + 1) * HB * 128])
            hst = hpool.tile([128, HB, CO + 4 * H], BF16, tag="hst")
            for i in range(HB):
                ps = pproj.tile([128, AL], F32)
                for k in range(KIN):
                    nc.tensor.matmul(
                        ps[:],
                        xt_t[:, k, i * 128:(i + 1) * 128],
                        w_t[:, k, :],
                        start=(k == 0), stop=(k == KIN - 1),
                    )
                nc.scalar.copy(hst[:, i, 0:CO], ps[:, 0:CO])
                nc.scalar.copy(hst[:, i, CO:CO + 4 * H].bitcast(F32),
                               ps[:, CO:AL])
            tv = htab[b * HB * 128:(b + 1) * HB * 128,
                      0:CO + 4 * H].rearrange("(t p) r -> p t r", p=128)
            nc.sync.dma_start(out=tv, in_=hst[:])

        # ---- edge phase ----
        # pre-snap the distinct num_idxs values (one register each, reused
        # by every dma_gather instead of allocating per call)
        nidx_val = {}
        for nv in sorted({L * PT for (_t, _c, _lo, L, _f, _l) in sched}):
            reg = nc.alloc_registers(engines=[mybir.EngineType.Pool])
            nc.regs_mov(reg, nv)
            nidx_val[nv] = nc.snap(reg, donate=True)
        ost = opool.tile([128, LT, CO], F32, tag="ost")
        pout = {}
        for r, (t, ch0, loc0, L, first, last) in enumerate(sched):
            gat = gpool.tile([128, GROUP, ROW], BF16, tag="gat")
            nc.gpsimd.dma_gather(
                out_ap=gat[:, 0:L, :],
                in_ap=htab[:],
                idxs_ap=gidx_t[:, ch0 * 8:(ch0 + L) * 8],
                num_idxs=L * PT,
                num_idxs_reg=nidx_val[L * PT],
                elem_size=ROW,
            )
            ga2 = gpool.tile([128, GROUP, ROW - CO], BF16, tag="ga2")
            nc.gpsimd.dma_gather(
                out_ap=ga2[:, 0:L, :],
                in_ap=htab[:, CO:ROW],
                idxs_ap=gidx2_t[:, ch0 * 8:(ch0 + L) * 8],
                num_idxs=L * PT,
                num_idxs_reg=nidx_val[L * PT],
                elem_size=ROW - CO,
                elem_step=ROW,
            )
            # edge-major one-hot for aggregation: [128e, L, 128n]
            oh = epool.tile([128, GROUP, 128], BF16, tag="oh")
            nc.vector.tensor_tensor(
                oh[:, 0:L, :],
                dstp_t[:, ch0:ch0 + L, :].to_broadcast([128, L, 128]),
                iotar_t[:].to_broadcast([128, L, 128]),
                OP.is_equal,
            )
            # ex = exp(leaky_relu(al_s[src] + al_d[dst])), logits in f32
            lg = epool.tile([128, GROUP, H], F32, tag="lg")
            nc.vector.tensor_tensor(
                lg[:, 0:L, :],
                gat[:, 0:L, CO:CO + 2 * H].bitcast(F32),
                ga2[:, 0:L, 2 * H:4 * H].bitcast(F32), OP.add)
            lr = epool.tile([128, GROUP, H], F32, tag="lr")
            nc.vector.scalar_tensor_tensor(
                lr[:, 0:L, :], lg[:, 0:L, :], NEG_SLOPE, lg[:, 0:L, :],
                OP.mult, OP.max)
            ex = epool.tile([128, GROUP, H, 1], F32, tag="ex")
            nc.scalar.activation(ex[:, 0:L, :, :], lr[:, 0:L, :], AF.Exp)
            # weighted moving rows [128e, L, CO+H]
            mov = epool.tile([128, GROUP, CO + H], BF16, tag="mov")
            if H == 1:
                for j in range(L):
                    nc.vector.tensor_scalar(
                        mov[:, j, 0:CO], gat[:, j, 0:CO],
                        ex[:, j, :, :], None, OP.mult)
            else:
                nc.vector.tensor_tensor(
                    mov[:, 0:L, 0:CO], gat[:, 0:L, 0:CO],
                    ex[:, 0:L, :, :].to_broadcast([128, L, H, CPH]),
                    OP.mult)
            nc.vector.tensor_copy(mov[:, 0:L, CO:CO + H], ex[:, 0:L, :, :])
            # aggregation into the tile's psum accumulator
            if first:
                pout[t] = pagg.tile([128, CO + H], F32, tag="pout",
                                    name=f"pout{t}")
            po = pout[t]
            for j in range(L):
                nc.tensor.matmul(
                    po[:], oh[:, j, :], mov[:, j, :],
                    start=(first and j == 0), stop=(last and j == L - 1))
            if last:
                rcp = epool.tile([128, H, 1], F32, tag="rcp")
                nc.vector.reciprocal(rcp[:, :, 0], po[:, CO:CO + H])
                od = epool.tile([128, CO], F32, tag="od")
                nc.vector.tensor_tensor(
                    od[:], po[:, 0:CO],
                    rcp[:].to_broadcast([128, H, CPH]), OP.mult)
                if relu:
                    tmp = epool.tile([128, CO], F32, tag="tmp")
                    nc.vector.tensor_tensor(tmp[:], od[:], bias_t[:], OP.add)
                    nc.vector.tensor_scalar_max(ost[:, t, :], tmp[:], 0.0)
                else:
                    nc.vector.tensor_tensor(ost[:, t, :], od[:], bias_t[:],
                                            OP.add)

        out_v = out_d[:].rearrange("(t p) c -> p t c", p=128)
        nc.sync.dma_start(out=out_v, in_=ost[:])

    nc.compile()
    return nc


# --------------------------------------------------------------------------
# host staging
# --------------------------------------------------------------------------
def stage_layer_inputs(cfg: Cfg, plan, x_full, W, att_src, att_dst, b):
    N, CO, H, AL, KIN = cfg.N, cfg.CO, cfg.H, cfg.AL, cfg.KIN
    xpad = np.zeros((cfg.NPAD, cfg.CH), np.float32)
    xpad[:N] = x_full
    xT = np.ascontiguousarray(xpad.T).astype(BF)

    C = CO // H
    A_src = np.zeros((CO, H), np.float32)
    A_dst = np.zeros((CO, H), np.float32)
    for h in range(H):
        A_src[h * C:(h + 1) * C, h] = att_src[h]
        A_dst[h * C:(h + 1) * C, h] = att_dst[h]
    Wf = np.asarray(W, np.float32)
    wfull = np.concatenate([Wf, Wf @ A_src, Wf @ A_dst], axis=1)  # [CH, AL]
    wext = np.ascontiguousarray(
        wfull.reshape(KIN, 128, AL).transpose(1, 0, 2)).astype(BF)

    bias_rep = np.tile(np.asarray(b, np.float32).reshape(1, CO), (128, 1))
    iotar = np.tile(np.arange(128, dtype=np.float32), (128, 1)).astype(BF)

    in_maps = []
    for c in range(cfg.NC):
        in_maps.append({
            "xT": xT,
            "wext": wext,
            "bias": bias_rep.astype(np.float32),
            "iotar": iotar,
            "gidx": plan["gidx"][c],
            "gidx2": plan["gidx2"][c],
            "dstp": plan["dstp"][c].astype(BF),
        })
    return in_maps


# --------------------------------------------------------------------------
# main entry
# --------------------------------------------------------------------------
_CACHE = {}
LAST_RESULTS = []


def kernel(x, edge_index, W1, att_src1, att_dst1, b1, W2, att_src2, att_dst2,
           b2):
    x = np.asarray(x, np.float32)
    ei = np.asarray(edge_index)
    N = x.shape[0]

    cfg1 = Cfg(N, 256, 256, 4, 8)
    cfg2 = Cfg(N, 256, 256, 1, 8)

    src = np.concatenate([ei[0], np.arange(N, dtype=np.int64)])
    dst = np.concatenate([ei[1], np.arange(N, dtype=np.int64)])
    plan = build_plan(cfg1, src, dst)

    key = ("progs", N)
    if key not in _CACHE:
        _CACHE[key] = (
            build_layer_program(cfg1, plan, relu=True),
            build_layer_program(cfg2, plan, relu=False),
        )
    nc1, nc2 = _CACHE[key]

    LAST_RESULTS.clear()
    in1 = stage_layer_inputs(cfg1, plan, x, W1, att_src1, att_dst1, b1)
    r1 = run_bass_kernel_spmd(nc1, in1, core_ids=list(range(8)))
    LAST_RESULTS.append(r1)
    x2 = np.concatenate([np.asarray(r1.results[c]["out"], np.float32)
                         for c in range(8)], axis=0)[:N]

    in2 = stage_layer_inputs(cfg2, plan, x2, W2, att_src2, att_dst2, b2)
    r2 = run_bass_kernel_spmd(nc2, in2, core_ids=list(range(8)))
    LAST_RESULTS.append(r2)
    out = np.concatenate([np.asarray(r2.results[c]["out"], np.float32)
                          for c in range(8)], axis=0)[:N]
    return out
